# revision 1
# baseline (speedup 1.0000x reference)
"""Trainium2 Bass kernel for CoOccurWithNorm.

Computes per-(image,channel) soft co-occurrence histograms of horizontally
adjacent pixel pairs, normalized by the per-histogram max.

Input  X: [64, 3, 512, 512] fp32, values in [0, 255)
Output:   [64, 3, 256, 256] fp32

Sharding: data-parallel over batch. Core k handles images [8k, 8k+8) ->
24 (image,channel) histograms per core. No cross-core communication.

Algorithm per (b,c):
  hist = sum_c W_c^T @ W_{c+1} over image columns c, accumulated in PSUM,
  where W_c is the [128 rows, 256 bins] soft one-hot (raised-cosine weights
  w0 = (1+cos(pi*f))/2 at bin ix=floor(x), w1 = 1-w0 at ix+1) of column c of
  a 128-row block. Each column's one-hot serves as rhs for chunk c-1 and
  lhsT for chunk c. One-hots are built sparsely by GPSIMD local_scatter
  (2 writes per sample) from precomputed weight/index pair streams.
  Normalization: hist / max(hist) on-device.
"""

import sys
import types
import numpy as np

sys.path.insert(0, "/root/.axon_site/_ro/trn_rl_repo")

import concourse.bass as bass
import concourse.bacc as bacc
import concourse.tile as tile
import concourse.mybir as mybir
import concourse.bass_isa as bass_isa

N_CORES = 8
NBINS = 256
H = 512
W = 512
PB = 128  # partition block (rows per block)
GRP = 7  # columns per local_scatter group (num_elems 7*256=1792, *32 < 2^16)

_PI = float(np.pi)


def install_ntff_hook():
    """Register the axon NTFF profiling hook (missing antenv.axon_hooks shim)."""
    import antenv

    if "antenv.axon_hooks" in sys.modules:
        return
    hooks_mod = types.ModuleType("antenv.axon_hooks")
    _hook = [None]
    hooks_mod.set_axon_ntff_profile_hook = lambda h: _hook.__setitem__(0, h)
    hooks_mod.get_axon_ntff_profile_hook = lambda: _hook[0]
    sys.modules["antenv.axon_hooks"] = hooks_mod
    antenv.axon_hooks = hooks_mod
    try:
        from trn_agent_boot.trn_boot import _ntff_profile_via_ctypes

        hooks_mod.set_axon_ntff_profile_hook(
            _ntff_profile_via_ctypes("/opt/axon/libaxon_pjrt.so")
        )
    except Exception:
        pass


def build_nc(n_bc=24, n_rb=4, debug=False):
    """Build the per-core Bass module.

    n_bc: number of (image,channel) histograms this core computes.
    n_rb: number of 128-row blocks per image (4 for H=512).
    """
    f32 = mybir.dt.float32
    bf16 = mybir.dt.bfloat16
    i16 = mybir.dt.int16

    nc = bacc.Bacc("TRN2", target_bir_lowering=False, debug=debug)

    n_rows = n_bc * n_rb * PB
    XS = nc.dram_tensor("XS", [n_rows, W], f32, kind="ExternalInput")
    OUT = nc.dram_tensor("OUT", [n_bc * NBINS, NBINS], f32, kind="ExternalOutput")

    with tile.TileContext(nc) as tc:
        with (
            tc.tile_pool(name="const", bufs=1) as const_pool,
            tc.tile_pool(name="xin", bufs=2) as xin_pool,
            tc.tile_pool(name="bld", bufs=2) as bld_pool,
            tc.tile_pool(name="wi", bufs=2) as wi_pool,
            tc.tile_pool(name="wt", bufs=3) as wt_pool,
            tc.tile_pool(name="dd", bufs=4) as dd_pool,
            tc.tile_pool(name="wtd", bufs=16) as wtd_pool,
            tc.tile_pool(name="ep", bufs=2) as ep_pool,
            tc.tile_pool(name="psum", bufs=2, space=bass.MemorySpace.PSUM) as psum_pool,
        ):
            # Constant index-offset pattern: for column-pair slot j in [0,1024):
            #   P[j] = 256*((j//2) % GRP) + (j % 2)
            n_grp_full = W // GRP  # full groups of GRP columns
            tail_cols = W - n_grp_full * GRP
            n_grp = n_grp_full + (1 if tail_cols else 0)
            ptile = const_pool.tile([128, n_grp * GRP * 2], i16)
            p4 = ptile[:].rearrange("p (a b t) -> p a b t", b=GRP, t=2)
            nc.gpsimd.iota(
                p4, pattern=[[0, n_grp], [NBINS, GRP], [1, 2]], base=0,
                channel_multiplier=0,
            )
            # bias constant for the Sin activation: cos(pi*f) = -sin(pi*f - pi/2),
            # keeping the Sin argument within the ScalarE table range [-pi, pi]
            sin_bias = const_pool.tile([128, 1], f32)
            nc.vector.memset(sin_bias[:], -_PI / 2.0)
            # dense-builder consts: cos(pi*u) = sin(pi/2 - pi*u) for u in [0,1]
            sin_bias_p = const_pool.tile([128, 1], f32)
            nc.vector.memset(sin_bias_p[:], _PI / 2.0)
            iot = const_pool.tile([128, NBINS], f32)
            nc.gpsimd.iota(iot[:], pattern=[[1, NBINS]], base=0,
                           channel_multiplier=0,
                           allow_small_or_imprecise_dtypes=True)

            with tc.For_i(0, n_bc, 1) as iv:
                epsum = [
                    psum_pool.tile([128, NBINS], f32, tag="eps0", name="eps0"),
                    psum_pool.tile([128, NBINS], f32, tag="eps1", name="eps1"),
                ]
                for rb in range(n_rb):
                    # ---- load one 128-row block of the image ----
                    xt = xin_pool.tile([128, W], f32)
                    nc.sync.dma_start(
                        xt[:], XS[bass.ds(iv * (n_rb * PB) + rb * PB, PB), :]
                    )
                    # ---- build weight + index pair streams ----
                    xc = bld_pool.tile([128, W], f32, tag="xc")
                    nc.vector.tensor_scalar(xc[:], xt[:], 254.999985, None,
                                            op0=mybir.AluOpType.min)
                    # floor/frac without `mod` (not a HW TensorScalar op):
                    # rn = round-to-nearest via the 2^23 magic number, then
                    # correct rn(x) > x cases to get floor exactly.
                    rn = bld_pool.tile([128, W], f32, tag="rn")
                    nc.vector.tensor_scalar(
                        rn[:], xc[:], 8388608.0, 8388608.0,
                        op0=mybir.AluOpType.add, op1=mybir.AluOpType.subtract,
                    )
                    fr0 = bld_pool.tile([128, W], f32, tag="fr0")
                    nc.vector.tensor_sub(fr0[:], xc[:], rn[:])
                    neg = bld_pool.tile([128, W], f32, tag="neg")
                    nc.vector.tensor_scalar(neg[:], fr0[:], 0.0, None,
                                            op0=mybir.AluOpType.is_lt)
                    fr = bld_pool.tile([128, W], f32, tag="fr")
                    nc.vector.tensor_add(fr[:], fr0[:], neg[:])
                    ixf = bld_pool.tile([128, W], f32, tag="ixf")
                    nc.vector.tensor_sub(ixf[:], xc[:], fr[:])
                    cosv = bld_pool.tile([128, W], f32, tag="cosv")
                    nc.scalar.activation(
                        cosv[:], fr[:], mybir.ActivationFunctionType.Sin,
                        bias=sin_bias[:], scale=_PI,
                    )
                    # interleaved (w0, w1) bf16 pairs
                    w01 = wi_pool.tile([128, 2 * W], bf16, tag="w01")
                    nc.vector.tensor_scalar(
                        w01[:, 0 : 2 * W : 2], cosv[:], -0.5, 0.5,
                        op0=mybir.AluOpType.mult, op1=mybir.AluOpType.add,
                    )
                    nc.vector.tensor_scalar(
                        w01[:, 1 : 2 * W : 2], cosv[:], 0.5, 0.5,
                        op0=mybir.AluOpType.mult, op1=mybir.AluOpType.add,
                    )
                    # interleaved (ix, ix) int16 pairs, then += P pattern
                    idx01 = wi_pool.tile([128, 2 * W], i16, tag="idx01")
                    nc.vector.tensor_scalar(
                        idx01[:, 0 : 2 * W : 2], ixf[:], 0.0, None,
                        op0=mybir.AluOpType.add,
                    )
                    nc.vector.tensor_scalar(
                        idx01[:, 1 : 2 * W : 2], ixf[:], 0.0, None,
                        op0=mybir.AluOpType.add,
                    )
                    nc.vector.tensor_tensor(
                        idx01[:], idx01[:], ptile[:, 0 : 2 * W], op=mybir.AluOpType.add
                    )

                    # ---- build one-hots (GPSIMD scatter or DVE dense) ----
                    colap = [None] * W
                    first = rb == 0
                    last = rb == n_rb - 1

                    def chunks_of(g, colap=colap, first=first, last=last):
                        # matmul chunks whose lhsT column lives in group g
                        for cc in range(GRP):
                            c = GRP * g + cc
                            if c >= W - 1:
                                break
                            st = first and c == 0
                            sp = last and c == W - 2
                            for h in range(2):
                                nc.tensor.matmul(
                                    epsum[h][:],
                                    colap[c][:, h * 128 : h * 128 + 128],
                                    colap[c + 1][:],
                                    start=st, stop=sp,
                                )

                    def build_group(g):
                        ncols = min(GRP, W - GRP * g)
                        if g % 5 == 4 and ncols == GRP:
                            # dense build on DVE + ScalarE (offloads GPSIMD):
                            # dc = clip(j - x, -1, 1); s = sin(pi/2*dc);
                            # W = 1 - s^2 = (1+cos(pi*dc))/2, exactly 0 outside support
                            for cc in range(ncols):
                                c = GRP * g + cc
                                dd = dd_pool.tile([128, NBINS], f32, tag="dd")
                                nc.vector.tensor_scalar(
                                    dd[:], iot[:], xc[:, c : c + 1], 1.0,
                                    op0=mybir.AluOpType.subtract,
                                    op1=mybir.AluOpType.min,
                                )
                                nc.vector.tensor_scalar(
                                    dd[:], dd[:], -1.0, None, op0=mybir.AluOpType.max
                                )
                                sv = dd_pool.tile([128, NBINS], f32, tag="sv")
                                nc.scalar.activation(
                                    sv[:], dd[:], mybir.ActivationFunctionType.Sin,
                                    scale=_PI / 2.0,
                                )
                                sq = dd_pool.tile([128, NBINS], f32, tag="sq")
                                nc.vector.tensor_mul(sq[:], sv[:], sv[:])
                                wd = wtd_pool.tile([128, NBINS], bf16, tag="wtd")
                                nc.vector.tensor_scalar(
                                    wd[:], sq[:], -1.0, 1.0,
                                    op0=mybir.AluOpType.mult, op1=mybir.AluOpType.add,
                                )
                                colap[c] = wd[:]
                        else:
                            wt = wt_pool.tile([128, GRP * NBINS], bf16, tag="wt")
                            nc.gpsimd.local_scatter(
                                wt[:],
                                w01[:, 2 * GRP * g : 2 * GRP * g + 2 * ncols],
                                idx01[:, 2 * GRP * g : 2 * GRP * g + 2 * ncols],
                                channels=128,
                                num_elems=GRP * NBINS,
                                num_idxs=2 * ncols,
                            )
                            for cc in range(ncols):
                                colap[GRP * g + cc] = wt[:, cc * NBINS : (cc + 1) * NBINS]

                    for g in range(n_grp):
                        build_group(g)
                        if g > 0:
                            chunks_of(g - 1)
                    chunks_of(n_grp - 1)

                # ---- epilogue: normalize by max and store ----
                mx = ep_pool.tile([128, 2], f32, tag="mx")
                for h in range(2):
                    nc.vector.tensor_reduce(
                        mx[:, h : h + 1], epsum[h][:],
                        axis=mybir.AxisListType.X, op=mybir.AluOpType.max,
                    )
                ar = ep_pool.tile([128, 2], f32, tag="ar")
                nc.gpsimd.partition_all_reduce(
                    ar[:], mx[:], channels=128, reduce_op=bass_isa.ReduceOp.max
                )
                vm128 = ep_pool.tile([128, 1], f32, tag="vm128")
                nc.vector.tensor_reduce(
                    vm128[:], ar[:], axis=mybir.AxisListType.X, op=mybir.AluOpType.max
                )
                rv128 = ep_pool.tile([128, 1], f32, tag="rv128")
                nc.vector.reciprocal(rv128[:], vm128[:])
                outs = ep_pool.tile([128, 2 * NBINS], f32, tag="outs")
                for h in range(2):
                    nc.vector.tensor_scalar(
                        outs[:, h * NBINS : (h + 1) * NBINS], epsum[h][:],
                        rv128[:], None, op0=mybir.AluOpType.mult,
                    )
                    nc.sync.dma_start(
                        OUT[bass.ds(iv * NBINS + h * 128, 128), :],
                        outs[:, h * NBINS : (h + 1) * NBINS],
                    )

    nc.compile()
    return nc


_NC_CACHE = {}


def _get_nc(key=(24, 4)):
    if key not in _NC_CACHE:
        _NC_CACHE[key] = build_nc(n_bc=key[0], n_rb=key[1], debug=False)
    return _NC_CACHE[key]


def kernel(X: np.ndarray) -> np.ndarray:
    """X: [64, 3, 512, 512] fp32 -> [64, 3, 256, 256] fp32."""
    from concourse.bass_utils import run_bass_kernel_spmd

    B, C, Hh, Ww = X.shape
    assert (Hh, Ww) == (H, W)
    per = B // N_CORES  # images per core
    n_bc = per * C

    nc = _get_nc((n_bc, H // PB))

    in_maps = []
    for k in range(N_CORES):
        shard = X[k * per : (k + 1) * per]  # [per, C, H, W]
        in_maps.append(
            {"XS": np.ascontiguousarray(shard.reshape(n_bc * H, W), dtype=np.float32)}
        )

    res = run_bass_kernel_spmd(nc, in_maps, core_ids=list(range(N_CORES)))
    out = np.empty((B, C, NBINS, NBINS), dtype=np.float32)
    for k in range(N_CORES):
        out[k * per : (k + 1) * per] = res.results[k]["OUT"].reshape(
            per, C, NBINS, NBINS
        )
    return out



# revision 2
# speedup vs baseline: 1.3119x; 1.3119x over previous
"""Trainium2 Bass kernel for CoOccurWithNorm.

Computes per-(image,channel) soft co-occurrence histograms of horizontally
adjacent pixel pairs, normalized by the per-histogram max.

Input  X: [64, 3, 512, 512] fp32, values in [0, 255)
Output:   [64, 3, 256, 256] fp32

Sharding: data-parallel over batch. Core k handles images [8k, 8k+8) ->
24 (image,channel) histograms per core. No cross-core communication.

Algorithm per (b,c):
  hist = sum_c W_c^T @ W_{c+1} over image columns c, accumulated in PSUM,
  where W_c is the [128 rows, 256 bins] soft one-hot (raised-cosine weights
  w0 = (1+cos(pi*f))/2 at bin ix=floor(x), w1 = 1-w0 at ix+1) of column c of
  a 128-row block. Each column's one-hot serves as rhs for chunk c-1 and
  lhsT for chunk c. One-hots are built either sparsely by GPSIMD
  local_scatter (2 writes per sample) from precomputed weight/index pair
  streams, or densely via w = sin^2(pi/2 * clip(i - x + 1, 0, 2)) split
  between DVE and ScalarE; the dense fraction is tuned so DVE/ScalarE/GPSIMD
  all stay under the TensorE roofline.
  Normalization is deferred: per-iv only PSUM->SBUF copy + free-dim max; one
  batched cross-partition max (partition_all_reduce) at the end so GPSIMD's
  scatter queue is never blocked mid-loop.
"""

import sys
import types
import numpy as np

sys.path.insert(0, "/root/.axon_site/_ro/trn_rl_repo")

import concourse.bass as bass
import concourse.bacc as bacc
import concourse.tile as tile
import concourse.mybir as mybir
import concourse.bass_isa as bass_isa

N_CORES = 8
NBINS = 256
H = 512
W = 512
PB = 128  # partition block (rows per block)
GRP = 7  # columns per local_scatter group (num_elems 7*256=1792, *32 < 2^16)

_PI = float(np.pi)


def install_ntff_hook():
    """Register the axon NTFF profiling hook (missing antenv.axon_hooks shim)."""
    import antenv

    if "antenv.axon_hooks" in sys.modules:
        return
    hooks_mod = types.ModuleType("antenv.axon_hooks")
    _hook = [None]
    hooks_mod.set_axon_ntff_profile_hook = lambda h: _hook.__setitem__(0, h)
    hooks_mod.get_axon_ntff_profile_hook = lambda: _hook[0]
    sys.modules["antenv.axon_hooks"] = hooks_mod
    antenv.axon_hooks = hooks_mod
    try:
        from trn_agent_boot.trn_boot import _ntff_profile_via_ctypes

        hooks_mod.set_axon_ntff_profile_hook(
            _ntff_profile_via_ctypes("/opt/axon/libaxon_pjrt.so")
        )
    except Exception:
        pass


def build_nc(n_bc=24, n_rb=4, debug=False):
    """Build the per-core Bass module.

    n_bc: number of (image,channel) histograms this core computes.
    n_rb: number of 128-row blocks per image (4 for H=512).
    """
    f32 = mybir.dt.float32
    bf16 = mybir.dt.bfloat16
    i16 = mybir.dt.int16

    nc = bacc.Bacc("TRN2", target_bir_lowering=False, debug=debug)

    n_rows = n_bc * n_rb * PB
    XS = nc.dram_tensor("XS", [n_rows, W], f32, kind="ExternalInput")
    OUT = nc.dram_tensor("OUT", [n_bc * NBINS, NBINS], f32, kind="ExternalOutput")

    n_grp_full = W // GRP  # full groups of GRP columns
    tail_cols = W - n_grp_full * GRP
    n_grp = n_grp_full + (1 if tail_cols else 0)
    # dense groups: spread evenly + force the ragged tail group dense
    dense_groups = set(g for g in range(n_grp) if g % 6 == 3)
    if tail_cols:
        dense_groups.add(n_grp - 1)

    with tile.TileContext(nc) as tc:
        with (
            tc.tile_pool(name="const", bufs=1) as const_pool,
            tc.tile_pool(name="hist", bufs=1) as hist_pool,
            tc.tile_pool(name="xin", bufs=2) as xin_pool,
            tc.tile_pool(name="bld", bufs=2) as bld_pool,
            tc.tile_pool(name="wi", bufs=2) as wi_pool,
            tc.tile_pool(name="wt", bufs=3) as wt_pool,
            tc.tile_pool(name="dd", bufs=4) as dd_pool,
            tc.tile_pool(name="wtd", bufs=16) as wtd_pool,
            tc.tile_pool(name="ep", bufs=2) as ep_pool,
            tc.tile_pool(name="psum", bufs=2, space=bass.MemorySpace.PSUM) as psum_pool,
        ):
            # Constant index-offset pattern: for column-pair slot j in [0,1024):
            #   P[j] = 256*((j//2) % GRP) + (j % 2)
            ptile = const_pool.tile([128, n_grp * GRP * 2], i16)
            p4 = ptile[:].rearrange("p (a b t) -> p a b t", b=GRP, t=2)
            nc.gpsimd.iota(
                p4, pattern=[[0, n_grp], [NBINS, GRP], [1, 2]], base=0,
                channel_multiplier=0,
            )
            # bias constant for the Sin activation: cos(pi*f) = -sin(pi*f - pi/2),
            # keeping the Sin argument within the ScalarE table range [-pi, pi]
            sin_bias = const_pool.tile([128, 1], f32)
            nc.vector.memset(sin_bias[:], -_PI / 2.0)
            iot = const_pool.tile([128, NBINS], f32)
            nc.gpsimd.iota(iot[:], pattern=[[1, NBINS]], base=0,
                           channel_multiplier=0,
                           allow_small_or_imprecise_dtypes=True)

            # unnormalized histograms + per-(iv,h) free-dim maxes, normalized
            # in one batched pass after the main loop
            hsb = hist_pool.tile([128, n_bc * 2 * NBINS], f32)
            mxt = hist_pool.tile([128, 2 * n_bc], f32)

            dense_ctr = [0]

            for iv in range(n_bc):
                epsum = [
                    psum_pool.tile([128, NBINS], f32, tag="eps0", name="eps0"),
                    psum_pool.tile([128, NBINS], f32, tag="eps1", name="eps1"),
                ]
                for rb in range(n_rb):
                    # ---- load one 128-row block of the image ----
                    xt = xin_pool.tile([128, W], f32)
                    nc.sync.dma_start(
                        xt[:], XS[bass.ds(iv * (n_rb * PB) + rb * PB, PB), :]
                    )
                    # ---- build weight + index pair streams ----
                    xc = bld_pool.tile([128, W], f32, tag="xc")
                    nc.vector.tensor_scalar(xc[:], xt[:], 254.999985, None,
                                            op0=mybir.AluOpType.min)
                    # x - 1 per column, used by the dense builder
                    xm1 = bld_pool.tile([128, W], f32, tag="xm1")
                    nc.vector.tensor_scalar(xm1[:], xc[:], 1.0, None,
                                            op0=mybir.AluOpType.subtract)
                    # floor/frac without `mod` (not a HW TensorScalar op):
                    # rn = round-to-nearest via the 2^23 magic number, then
                    # correct rn(x) > x cases to get floor exactly.
                    rn = bld_pool.tile([128, W], f32, tag="rn")
                    nc.vector.tensor_scalar(
                        rn[:], xc[:], 8388608.0, 8388608.0,
                        op0=mybir.AluOpType.add, op1=mybir.AluOpType.subtract,
                    )
                    fr0 = bld_pool.tile([128, W], f32, tag="fr0")
                    nc.vector.tensor_sub(fr0[:], xc[:], rn[:])
                    neg = bld_pool.tile([128, W], f32, tag="neg")
                    nc.vector.tensor_scalar(neg[:], fr0[:], 0.0, None,
                                            op0=mybir.AluOpType.is_lt)
                    fr = bld_pool.tile([128, W], f32, tag="fr")
                    nc.vector.tensor_add(fr[:], fr0[:], neg[:])
                    ixf = bld_pool.tile([128, W], f32, tag="ixf")
                    nc.vector.tensor_sub(ixf[:], xc[:], fr[:])
                    cosv = bld_pool.tile([128, W], f32, tag="cosv")
                    nc.scalar.activation(
                        cosv[:], fr[:], mybir.ActivationFunctionType.Sin,
                        bias=sin_bias[:], scale=_PI,
                    )
                    # interleaved (w0, w1) bf16 pairs
                    w01 = wi_pool.tile([128, 2 * W], bf16, tag="w01")
                    nc.vector.tensor_scalar(
                        w01[:, 0 : 2 * W : 2], cosv[:], -0.5, 0.5,
                        op0=mybir.AluOpType.mult, op1=mybir.AluOpType.add,
                    )
                    nc.vector.tensor_scalar(
                        w01[:, 1 : 2 * W : 2], cosv[:], 0.5, 0.5,
                        op0=mybir.AluOpType.mult, op1=mybir.AluOpType.add,
                    )
                    # interleaved (ix, ix) int16 pairs, then += P pattern
                    idx01 = wi_pool.tile([128, 2 * W], i16, tag="idx01")
                    nc.vector.tensor_scalar(
                        idx01[:, 0 : 2 * W : 2], ixf[:], 0.0, None,
                        op0=mybir.AluOpType.add,
                    )
                    nc.vector.tensor_scalar(
                        idx01[:, 1 : 2 * W : 2], ixf[:], 0.0, None,
                        op0=mybir.AluOpType.add,
                    )
                    nc.vector.tensor_tensor(
                        idx01[:], idx01[:], ptile[:, 0 : 2 * W], op=mybir.AluOpType.add
                    )

                    # ---- build one-hots (GPSIMD scatter or DVE+ScalarE dense) ----
                    colap = [None] * W
                    first = rb == 0
                    last = rb == n_rb - 1

                    def chunks_of(g, colap=colap, first=first, last=last):
                        # matmul chunks whose lhsT column lives in group g
                        for cc in range(GRP):
                            c = GRP * g + cc
                            if c >= W - 1:
                                break
                            st = first and c == 0
                            sp = last and c == W - 2
                            for h in range(2):
                                nc.tensor.matmul(
                                    epsum[h][:],
                                    colap[c][:, h * 128 : h * 128 + 128],
                                    colap[c + 1][:],
                                    start=st, stop=sp,
                                )

                    def build_group(g, colap=colap, xm1=xm1, w01=w01, idx01=idx01):
                        ncols = min(GRP, W - GRP * g)
                        if g in dense_groups:
                            # dense build: t = clip(i - x + 1, 0, 2);
                            # w = sin^2(pi/2 * t), exactly 0 outside support.
                            # Variant S puts the square on ScalarE, variant D
                            # on DVE; mixed 2:1 to balance the two engines.
                            for cc in range(ncols):
                                c = GRP * g + cc
                                dense_ctr[0] += 1
                                use_dve_sq = dense_ctr[0] % 3 == 0
                                dd = dd_pool.tile([128, NBINS], f32, tag="dd")
                                nc.vector.tensor_scalar(
                                    dd[:], iot[:], xm1[:, c : c + 1], 2.0,
                                    op0=mybir.AluOpType.subtract,
                                    op1=mybir.AluOpType.min,
                                )
                                dc = dd_pool.tile([128, NBINS], f32, tag="dc")
                                nc.vector.tensor_scalar(
                                    dc[:], dd[:], 0.0, None, op0=mybir.AluOpType.max
                                )
                                sv = dd_pool.tile([128, NBINS], f32, tag="sv")
                                nc.scalar.activation(
                                    sv[:], dc[:], mybir.ActivationFunctionType.Sin,
                                    scale=_PI / 2.0,
                                )
                                wd = wtd_pool.tile([128, NBINS], bf16, tag="wtd")
                                if use_dve_sq:
                                    nc.vector.tensor_mul(wd[:], sv[:], sv[:])
                                else:
                                    nc.scalar.activation(
                                        wd[:], sv[:],
                                        mybir.ActivationFunctionType.Square,
                                    )
                                colap[c] = wd[:]
                        else:
                            wt = wt_pool.tile([128, GRP * NBINS], bf16, tag="wt")
                            nc.gpsimd.local_scatter(
                                wt[:],
                                w01[:, 2 * GRP * g : 2 * GRP * g + 2 * ncols],
                                idx01[:, 2 * GRP * g : 2 * GRP * g + 2 * ncols],
                                channels=128,
                                num_elems=GRP * NBINS,
                                num_idxs=2 * ncols,
                            )
                            for cc in range(ncols):
                                colap[GRP * g + cc] = wt[:, cc * NBINS : (cc + 1) * NBINS]

                    for g in range(n_grp):
                        build_group(g)
                        if g > 0:
                            chunks_of(g - 1)
                    chunks_of(n_grp - 1)

                # ---- per-iv epilogue: free-dim max + copy PSUM out ----
                for h in range(2):
                    nc.vector.tensor_reduce(
                        mxt[:, 2 * iv + h : 2 * iv + h + 1], epsum[h][:],
                        axis=mybir.AxisListType.X, op=mybir.AluOpType.max,
                    )
                    nc.vector.tensor_scalar(
                        hsb[:, (2 * iv + h) * NBINS : (2 * iv + h + 1) * NBINS],
                        epsum[h][:], 0.0, None, op0=mybir.AluOpType.add,
                    )

            # ---- batched normalization epilogue ----
            ar = ep_pool.tile([128, 2 * n_bc], f32, tag="ar")
            nc.gpsimd.partition_all_reduce(
                ar[:], mxt[:], channels=128, reduce_op=bass_isa.ReduceOp.max
            )
            for iv in range(n_bc):
                vm = ep_pool.tile([128, 1], f32, tag="vm")
                nc.vector.tensor_reduce(
                    vm[:], ar[:, 2 * iv : 2 * iv + 2],
                    axis=mybir.AxisListType.X, op=mybir.AluOpType.max,
                )
                rv = ep_pool.tile([128, 1], f32, tag="rv")
                nc.vector.reciprocal(rv[:], vm[:])
                outs = ep_pool.tile([128, 2 * NBINS], f32, tag="outs")
                for h in range(2):
                    nc.vector.tensor_scalar(
                        outs[:, h * NBINS : (h + 1) * NBINS],
                        hsb[:, (2 * iv + h) * NBINS : (2 * iv + h + 1) * NBINS],
                        rv[:], None, op0=mybir.AluOpType.mult,
                    )
                    nc.sync.dma_start(
                        OUT[bass.ds(iv * NBINS + h * 128, 128), :],
                        outs[:, h * NBINS : (h + 1) * NBINS],
                    )

    nc.compile()
    return nc


_NC_CACHE = {}


def _get_nc(key=(24, 4)):
    if key not in _NC_CACHE:
        _NC_CACHE[key] = build_nc(n_bc=key[0], n_rb=key[1], debug=False)
    return _NC_CACHE[key]


def kernel(X: np.ndarray) -> np.ndarray:
    """X: [64, 3, 512, 512] fp32 -> [64, 3, 256, 256] fp32."""
    from concourse.bass_utils import run_bass_kernel_spmd

    B, C, Hh, Ww = X.shape
    assert (Hh, Ww) == (H, W)
    per = B // N_CORES  # images per core
    n_bc = per * C

    nc = _get_nc((n_bc, H // PB))

    in_maps = []
    for k in range(N_CORES):
        shard = X[k * per : (k + 1) * per]  # [per, C, H, W]
        in_maps.append(
            {"XS": np.ascontiguousarray(shard.reshape(n_bc * H, W), dtype=np.float32)}
        )

    res = run_bass_kernel_spmd(nc, in_maps, core_ids=list(range(N_CORES)))
    out = np.empty((B, C, NBINS, NBINS), dtype=np.float32)
    for k in range(N_CORES):
        out[k * per : (k + 1) * per] = res.results[k]["OUT"].reshape(
            per, C, NBINS, NBINS
        )
    return out


# revision 3
# speedup vs baseline: 1.3188x; 1.0053x over previous
"""Trainium2 Bass kernel for CoOccurWithNorm.

Computes per-(image,channel) soft co-occurrence histograms of horizontally
adjacent pixel pairs, normalized by the per-histogram max.

Input  X: [64, 3, 512, 512] fp32, values in [0, 255)
Output:   [64, 3, 256, 256] fp32

Sharding: data-parallel over batch. Core k handles images [8k, 8k+8) ->
24 (image,channel) histograms per core. No cross-core communication.

Algorithm per (b,c):
  hist = sum_c W_c^T @ W_{c+1} over image columns c, accumulated in PSUM,
  where W_c is the [128 rows, 256 bins] soft one-hot (raised-cosine weights
  w0 = (1+cos(pi*f))/2 at bin ix=floor(x), w1 = 1-w0 at ix+1) of column c of
  a 128-row block. Each column's one-hot serves as rhs for chunk c-1 and
  lhsT for chunk c. One-hots are built either sparsely by GPSIMD
  local_scatter (2 writes per sample) from precomputed weight/index pair
  streams, or densely via w = sin^2(pi/2 * clip(i - x + 1, 0, 2)) split
  between DVE and ScalarE; the dense fraction is tuned so DVE/ScalarE/GPSIMD
  all stay under the TensorE roofline.
  Normalization is deferred: per-iv only PSUM->SBUF copy + free-dim max; one
  batched cross-partition max (partition_all_reduce) at the end so GPSIMD's
  scatter queue is never blocked mid-loop.
"""

import sys
import types
import numpy as np

sys.path.insert(0, "/root/.axon_site/_ro/trn_rl_repo")

import concourse.bass as bass
import concourse.bacc as bacc
import concourse.tile as tile
import concourse.mybir as mybir
import concourse.bass_isa as bass_isa

N_CORES = 8
NBINS = 256
H = 512
W = 512
PB = 128  # partition block (rows per block)
GRP = 7  # columns per local_scatter group (num_elems 7*256=1792, *32 < 2^16)

_PI = float(np.pi)


def install_ntff_hook():
    """Register the axon NTFF profiling hook (missing antenv.axon_hooks shim)."""
    import antenv

    if "antenv.axon_hooks" in sys.modules:
        return
    hooks_mod = types.ModuleType("antenv.axon_hooks")
    _hook = [None]
    hooks_mod.set_axon_ntff_profile_hook = lambda h: _hook.__setitem__(0, h)
    hooks_mod.get_axon_ntff_profile_hook = lambda: _hook[0]
    sys.modules["antenv.axon_hooks"] = hooks_mod
    antenv.axon_hooks = hooks_mod
    try:
        from trn_agent_boot.trn_boot import _ntff_profile_via_ctypes

        hooks_mod.set_axon_ntff_profile_hook(
            _ntff_profile_via_ctypes("/opt/axon/libaxon_pjrt.so")
        )
    except Exception:
        pass


def build_nc(n_bc=24, n_rb=4, debug=False):
    """Build the per-core Bass module.

    n_bc: number of (image,channel) histograms this core computes.
    n_rb: number of 128-row blocks per image (4 for H=512).
    """
    f32 = mybir.dt.float32
    bf16 = mybir.dt.bfloat16
    i16 = mybir.dt.int16

    nc = bacc.Bacc("TRN2", target_bir_lowering=False, debug=debug)

    n_rows = n_bc * n_rb * PB
    XS = nc.dram_tensor("XS", [n_rows, W], f32, kind="ExternalInput")
    OUT = nc.dram_tensor("OUT", [n_bc * NBINS, NBINS], f32, kind="ExternalOutput")

    n_grp_full = W // GRP  # full groups of GRP columns
    tail_cols = W - n_grp_full * GRP
    n_grp = n_grp_full + (1 if tail_cols else 0)
    # dense groups: spread evenly + force the ragged tail group dense.
    # ~20% dense keeps GPSIMD (the scatter path) comfortably under the
    # TensorE roofline while DVE/ScalarE absorb the dense work.
    dense_groups = set(g for g in range(n_grp) if g % 5 == 2)
    if tail_cols:
        dense_groups.add(n_grp - 1)

    with tile.TileContext(nc) as tc:
        with (
            tc.tile_pool(name="const", bufs=1) as const_pool,
            tc.tile_pool(name="hist", bufs=1) as hist_pool,
            tc.tile_pool(name="xin", bufs=2) as xin_pool,
            tc.tile_pool(name="bld", bufs=2) as bld_pool,
            tc.tile_pool(name="wi", bufs=2) as wi_pool,
            tc.tile_pool(name="wt", bufs=3) as wt_pool,
            tc.tile_pool(name="dd", bufs=4) as dd_pool,
            tc.tile_pool(name="wtd", bufs=16) as wtd_pool,
            tc.tile_pool(name="ep", bufs=2) as ep_pool,
            tc.tile_pool(name="psum", bufs=2, space=bass.MemorySpace.PSUM) as psum_pool,
        ):
            # Constant index-offset pattern: for column-pair slot j in [0,1024):
            #   P[j] = 256*((j//2) % GRP) + (j % 2)
            ptile = const_pool.tile([128, n_grp * GRP * 2], i16)
            p4 = ptile[:].rearrange("p (a b t) -> p a b t", b=GRP, t=2)
            nc.gpsimd.iota(
                p4, pattern=[[0, n_grp], [NBINS, GRP], [1, 2]], base=0,
                channel_multiplier=0,
            )
            # bias constant for the Sin activation: cos(pi*f) = -sin(pi*f - pi/2),
            # keeping the Sin argument within the ScalarE table range [-pi, pi]
            sin_bias = const_pool.tile([128, 1], f32)
            nc.vector.memset(sin_bias[:], -_PI / 2.0)
            iot = const_pool.tile([128, NBINS], f32)
            nc.gpsimd.iota(iot[:], pattern=[[1, NBINS]], base=0,
                           channel_multiplier=0,
                           allow_small_or_imprecise_dtypes=True)

            # unnormalized histograms + per-(iv,h) free-dim maxes, normalized
            # in one batched pass after the main loop
            hsb = hist_pool.tile([128, n_bc * 2 * NBINS], f32)
            mxt = hist_pool.tile([128, 2 * n_bc], f32)

            dense_ctr = [0]

            for iv in range(n_bc):
                epsum = [
                    psum_pool.tile([128, NBINS], f32, tag="eps0", name="eps0"),
                    psum_pool.tile([128, NBINS], f32, tag="eps1", name="eps1"),
                ]
                for rb in range(n_rb):
                    # ---- load one 128-row block of the image ----
                    xt = xin_pool.tile([128, W], f32)
                    nc.sync.dma_start(
                        xt[:], XS[bass.ds(iv * (n_rb * PB) + rb * PB, PB), :]
                    )
                    # ---- build weight + index pair streams ----
                    xc = bld_pool.tile([128, W], f32, tag="xc")
                    nc.vector.tensor_scalar(xc[:], xt[:], 254.999985, None,
                                            op0=mybir.AluOpType.min)
                    # x - 1 per column, used by the dense builder
                    xm1 = bld_pool.tile([128, W], f32, tag="xm1")
                    nc.vector.tensor_scalar(xm1[:], xc[:], 1.0, None,
                                            op0=mybir.AluOpType.subtract)
                    # floor/frac without `mod` (not a HW TensorScalar op):
                    # rn = round-to-nearest via the 2^23 magic number, then
                    # correct rn(x) > x cases to get floor exactly.
                    rn = bld_pool.tile([128, W], f32, tag="rn")
                    nc.vector.tensor_scalar(
                        rn[:], xc[:], 8388608.0, 8388608.0,
                        op0=mybir.AluOpType.add, op1=mybir.AluOpType.subtract,
                    )
                    fr0 = bld_pool.tile([128, W], f32, tag="fr0")
                    nc.vector.tensor_sub(fr0[:], xc[:], rn[:])
                    neg = bld_pool.tile([128, W], f32, tag="neg")
                    nc.vector.tensor_scalar(neg[:], fr0[:], 0.0, None,
                                            op0=mybir.AluOpType.is_lt)
                    fr = bld_pool.tile([128, W], f32, tag="fr")
                    nc.vector.tensor_add(fr[:], fr0[:], neg[:])
                    ixf = bld_pool.tile([128, W], f32, tag="ixf")
                    nc.vector.tensor_sub(ixf[:], xc[:], fr[:])
                    cosv = bld_pool.tile([128, W], f32, tag="cosv")
                    nc.scalar.activation(
                        cosv[:], fr[:], mybir.ActivationFunctionType.Sin,
                        bias=sin_bias[:], scale=_PI,
                    )
                    # interleaved (w0, w1) bf16 pairs
                    w01 = wi_pool.tile([128, 2 * W], bf16, tag="w01")
                    nc.vector.tensor_scalar(
                        w01[:, 0 : 2 * W : 2], cosv[:], -0.5, 0.5,
                        op0=mybir.AluOpType.mult, op1=mybir.AluOpType.add,
                    )
                    nc.vector.tensor_scalar(
                        w01[:, 1 : 2 * W : 2], cosv[:], 0.5, 0.5,
                        op0=mybir.AluOpType.mult, op1=mybir.AluOpType.add,
                    )
                    # interleaved (ix, ix) int16 pairs, then += P pattern
                    idx01 = wi_pool.tile([128, 2 * W], i16, tag="idx01")
                    nc.vector.tensor_scalar(
                        idx01[:, 0 : 2 * W : 2], ixf[:], 0.0, None,
                        op0=mybir.AluOpType.add,
                    )
                    nc.vector.tensor_scalar(
                        idx01[:, 1 : 2 * W : 2], ixf[:], 0.0, None,
                        op0=mybir.AluOpType.add,
                    )
                    nc.vector.tensor_tensor(
                        idx01[:], idx01[:], ptile[:, 0 : 2 * W], op=mybir.AluOpType.add
                    )

                    # ---- build one-hots (GPSIMD scatter or DVE+ScalarE dense) ----
                    colap = [None] * W
                    first = rb == 0
                    last = rb == n_rb - 1

                    def chunks_of(g, colap=colap, first=first, last=last):
                        # matmul chunks whose lhsT column lives in group g
                        for cc in range(GRP):
                            c = GRP * g + cc
                            if c >= W - 1:
                                break
                            st = first and c == 0
                            sp = last and c == W - 2
                            for h in range(2):
                                nc.tensor.matmul(
                                    epsum[h][:],
                                    colap[c][:, h * 128 : h * 128 + 128],
                                    colap[c + 1][:],
                                    start=st, stop=sp,
                                )

                    def build_group(g, colap=colap, xm1=xm1, w01=w01, idx01=idx01):
                        ncols = min(GRP, W - GRP * g)
                        if g in dense_groups:
                            # dense build: t = clip(i - x + 1, 0, 2);
                            # w = sin^2(pi/2 * t), exactly 0 outside support.
                            # Variant S puts the square on ScalarE, variant D
                            # on DVE; mixed 2:1 to balance the two engines.
                            for cc in range(ncols):
                                c = GRP * g + cc
                                dense_ctr[0] += 1
                                use_dve_sq = dense_ctr[0] % 3 == 0
                                dd = dd_pool.tile([128, NBINS], f32, tag="dd")
                                nc.vector.tensor_scalar(
                                    dd[:], iot[:], xm1[:, c : c + 1], 2.0,
                                    op0=mybir.AluOpType.subtract,
                                    op1=mybir.AluOpType.min,
                                )
                                dc = dd_pool.tile([128, NBINS], f32, tag="dc")
                                nc.vector.tensor_scalar(
                                    dc[:], dd[:], 0.0, None, op0=mybir.AluOpType.max
                                )
                                sv = dd_pool.tile([128, NBINS], f32, tag="sv")
                                nc.scalar.activation(
                                    sv[:], dc[:], mybir.ActivationFunctionType.Sin,
                                    scale=_PI / 2.0,
                                )
                                wd = wtd_pool.tile([128, NBINS], bf16, tag="wtd")
                                if use_dve_sq:
                                    nc.vector.tensor_mul(wd[:], sv[:], sv[:])
                                else:
                                    nc.scalar.activation(
                                        wd[:], sv[:],
                                        mybir.ActivationFunctionType.Square,
                                    )
                                colap[c] = wd[:]
                        else:
                            wt = wt_pool.tile([128, GRP * NBINS], bf16, tag="wt")
                            nc.gpsimd.local_scatter(
                                wt[:],
                                w01[:, 2 * GRP * g : 2 * GRP * g + 2 * ncols],
                                idx01[:, 2 * GRP * g : 2 * GRP * g + 2 * ncols],
                                channels=128,
                                num_elems=GRP * NBINS,
                                num_idxs=2 * ncols,
                            )
                            for cc in range(ncols):
                                colap[GRP * g + cc] = wt[:, cc * NBINS : (cc + 1) * NBINS]

                    for g in range(n_grp):
                        build_group(g)
                        if g > 0:
                            chunks_of(g - 1)
                    chunks_of(n_grp - 1)

                # ---- per-iv epilogue: free-dim max + copy PSUM out ----
                for h in range(2):
                    nc.vector.tensor_reduce(
                        mxt[:, 2 * iv + h : 2 * iv + h + 1], epsum[h][:],
                        axis=mybir.AxisListType.X, op=mybir.AluOpType.max,
                    )
                    nc.vector.tensor_scalar(
                        hsb[:, (2 * iv + h) * NBINS : (2 * iv + h + 1) * NBINS],
                        epsum[h][:], 0.0, None, op0=mybir.AluOpType.add,
                    )

            # ---- batched normalization epilogue ----
            ar = ep_pool.tile([128, 2 * n_bc], f32, tag="ar")
            nc.gpsimd.partition_all_reduce(
                ar[:], mxt[:], channels=128, reduce_op=bass_isa.ReduceOp.max
            )
            for iv in range(n_bc):
                vm = ep_pool.tile([128, 1], f32, tag="vm")
                nc.vector.tensor_reduce(
                    vm[:], ar[:, 2 * iv : 2 * iv + 2],
                    axis=mybir.AxisListType.X, op=mybir.AluOpType.max,
                )
                rv = ep_pool.tile([128, 1], f32, tag="rv")
                nc.vector.reciprocal(rv[:], vm[:])
                outs = ep_pool.tile([128, 2 * NBINS], f32, tag="outs")
                for h in range(2):
                    nc.vector.tensor_scalar(
                        outs[:, h * NBINS : (h + 1) * NBINS],
                        hsb[:, (2 * iv + h) * NBINS : (2 * iv + h + 1) * NBINS],
                        rv[:], None, op0=mybir.AluOpType.mult,
                    )
                    nc.sync.dma_start(
                        OUT[bass.ds(iv * NBINS + h * 128, 128), :],
                        outs[:, h * NBINS : (h + 1) * NBINS],
                    )

    nc.compile()
    return nc


_NC_CACHE = {}


def _get_nc(key=(24, 4)):
    if key not in _NC_CACHE:
        _NC_CACHE[key] = build_nc(n_bc=key[0], n_rb=key[1], debug=False)
    return _NC_CACHE[key]


def kernel(X: np.ndarray) -> np.ndarray:
    """X: [64, 3, 512, 512] fp32 -> [64, 3, 256, 256] fp32."""
    from concourse.bass_utils import run_bass_kernel_spmd

    B, C, Hh, Ww = X.shape
    assert (Hh, Ww) == (H, W)
    per = B // N_CORES  # images per core
    n_bc = per * C

    nc = _get_nc((n_bc, H // PB))

    in_maps = []
    for k in range(N_CORES):
        shard = X[k * per : (k + 1) * per]  # [per, C, H, W]
        in_maps.append(
            {"XS": np.ascontiguousarray(shard.reshape(n_bc * H, W), dtype=np.float32)}
        )

    res = run_bass_kernel_spmd(nc, in_maps, core_ids=list(range(N_CORES)))
    out = np.empty((B, C, NBINS, NBINS), dtype=np.float32)
    for k in range(N_CORES):
        out[k * per : (k + 1) * per] = res.results[k]["OUT"].reshape(
            per, C, NBINS, NBINS
        )
    return out
